# revision 22
# baseline (speedup 1.0000x reference)
"""Trainium2 Bass kernel for nn_CryptoGNN (2-layer GCN + pooled heads).

Math notes (full derivation validated against the reference):
  With A = normalized adjacency (incl. self loops), P = [B,N] pooling matrix,
  cnt = nodes per graph:
    h1 = relu((A @ x) @ W1 + b1)
    P @ h2 = (PA @ h1) @ W2 + cnt*b2 + P @ h1        (layer 2 fully collapsed)
  where PA = P @ A is a dense [B, N] matrix computable from the integer
  graph structure alone.  Only ax = A @ x requires true sparse message
  passing on device; everything else is dense matmul.

Sharding: nodes (and the edges pointing at them) are split into 8
contiguous shards of 12544; each of the 8 NeuronCores independently
computes its shard's ax -> h1 -> partial G = [PA;P](shard)^T @ h1(shard)
([128,128]).  No collectives: the host sums the 8 partial Gs and runs the
tiny [64,*] head in numpy (microseconds).

Device phase A (per core) — sparse ax = A@x via GPSIMD ap_gather:
  * feature-transposed table: partition 16g+r holds feature r of node chunk
    g, scaled by dis[src] on device (one DVE mul)
  * ap_gather #1: per-group dst-sorted edge stream of src columns
  * fp32 prefix scan along the stream (tensor_tensor_scan)
  * ap_gather #2 of per-dst boundary columns + shifted difference
    -> per-group segment sums; folded across groups with one small
    PE matmul (selection matrix).
  +b1 folds into phase B's matmul-1 via an augmented sqrt(deg) row;
  dis[dst] folds into the host-built papt columns (relu(dis*z)=dis*relu(z)).

Device phase B (per core): 98 node tiles of 128 (papt streamed in 12-tile
slab DMAs, relu batched 4 tiles wide):
  mm1: z_t = axTaug_t @ W1aug                      [128,128] PSUM
  mm2: G += papt'_t^T @ relu(z_t)  (PSUM accumulated into one [128,128])
"""

import sys

if "/opt/trn_rl_repo" not in sys.path:
    sys.path.insert(0, "/opt/trn_rl_repo")

import numpy as np

N = 100000
E = 600000
B = 64
IN = 6
H = 128
S = 16

NSHARD = 12544            # nodes per core shard / per table chunk (98*128)
NG = 8                    # groups (= src chunks = cores)
NPAD = NSHARD * NG        # 100352
NE = NSHARD + 1           # table columns per group (+ zero column)
ND = NSHARD
NB = 12560                # boundary gather count: 1 + 12544 + 15  (%16==0)
NT = NSHARD // 128        # 98 node tiles per shard
P128 = 128

_compiled = {}


def _build_nc(JW):
    import concourse.bacc as bacc
    import concourse.mybir as mybir
    from concourse import tile

    f32 = mybir.dt.float32
    i16 = mybir.dt.int16

    nc = bacc.Bacc("TRN2", target_bir_lowering=False, debug=False)

    xt48 = nc.declare_dram_parameter("xt48", [48, NSHARD], f32, isOutput=False)
    dis_tab = nc.declare_dram_parameter("dis_tab", [NG, NE], f32, isOutput=False)
    gidx = nc.declare_dram_parameter("gidx", [P128, JW // 16], i16, isOutput=False)
    bidx = nc.declare_dram_parameter("bidx", [P128, NB // 16], i16, isOutput=False)
    sq = nc.declare_dram_parameter("sq", [1, NSHARD], f32, isOutput=False)
    papt = nc.declare_dram_parameter("papt", [NSHARD, P128], f32, isOutput=False)
    w1aug = nc.declare_dram_parameter("w1aug", [7, H], f32, isOutput=False)
    sel = nc.declare_dram_parameter("sel", [P128, 6], f32, isOutput=False)
    zrow = nc.declare_dram_parameter("zrow", [1, NE], f32, isOutput=False)
    selfsel = nc.declare_dram_parameter("selfsel", [P128, 6], f32, isOutput=False)
    gout = nc.declare_dram_parameter("gout", [P128, P128], f32, isOutput=True)

    with tile.TileContext(nc) as tc:
        with (
            tc.tile_pool(name="big", bufs=1) as big,
            tc.tile_pool(name="small", bufs=1) as small,
            tc.tile_pool(name="pstream", bufs=2) as pstream,
            tc.tile_pool(name="hbuf", bufs=3) as hbuf,
            tc.tile_pool(name="ps1", bufs=2, space="PSUM") as ps1p,
            tc.tile_pool(name="psA", bufs=2, space="PSUM") as psAp,
            tc.tile_pool(name="psG", bufs=1, space="PSUM") as psGp,
        ):
            # ---------- constants / small inputs ----------
            sel_t = small.tile([P128, 6], f32)
            nc.sync.dma_start(out=sel_t[:], in_=sel[:])
            selfsel_t = small.tile([P128, 6], f32)
            nc.sync.dma_start(out=selfsel_t[:], in_=selfsel[:])
            w1_t = small.tile([7, H], f32)
            nc.sync.dma_start(out=w1_t[:], in_=w1aug[:])
            gidx_t = small.tile([P128, JW // 16], i16)
            nc.sync.dma_start(out=gidx_t[:], in_=gidx[:])
            bidx_t = small.tile([P128, NB // 16], i16)
            nc.sync.dma_start(out=bidx_t[:], in_=bidx[:])

            # axTaug rows: 0-5 features (written by fold), 6 = sqrt(deg)
            axTaug = small.tile([7, NSHARD], f32)
            nc.sync.dma_start(out=axTaug[6:7, :], in_=sq[:])

            # ---------- phase A: table build ----------
            table = big.tile([P128, NE], f32, tag="t1")
            # zero the unused rows (r>=6 of each group) + the zero column via
            # DMA broadcasts (overlaps with the data loads; avoids a 13us
            # DVE memset on the critical path)
            for g in range(NG):
                nc.sync.dma_start(
                    out=table[16 * g + 6:16 * (g + 1), :],
                    in_=zrow[0:1, :].to_broadcast([10, NE]),
                )
            nc.vector.memset(table[:, NSHARD:NE], 0.0)
            for g in range(NG):
                nc.sync.dma_start(
                    out=table[16 * g:16 * g + 6, 0:NSHARD],
                    in_=xt48[6 * g:6 * g + 6, :],
                )
            disrep = big.tile([P128, NE], f32, tag="t2")
            for g in range(NG):
                nc.sync.dma_start(
                    out=disrep[16 * g:16 * (g + 1), :],
                    in_=dis_tab[g:g + 1, :].to_broadcast([16, NE]),
                )
            nc.vector.tensor_mul(table[:], table[:], disrep[:])

            # ---------- phase A: gather / scan / gather / diff ----------
            gath = big.tile([P128, JW], f32, tag="t3")
            nc.gpsimd.ap_gather(
                out_ap=gath[:], in_ap=table[:], idxs_ap=gidx_t[:],
                channels=P128, num_elems=NE, d=1, num_idxs=JW,
            )
            nc.vector.tensor_tensor_scan(
                out=gath[:], data0=gath[:], data1=gath[:], initial=0.0,
                op0=mybir.AluOpType.add, op1=mybir.AluOpType.bypass,
            )
            bnd = big.tile([P128, NB], f32, tag="t2")
            nc.gpsimd.ap_gather(
                out_ap=bnd[:], in_ap=gath[:], idxs_ap=bidx_t[:],
                channels=P128, num_elems=JW, d=1, num_idxs=NB,
            )
            # shifted difference, in place over bnd (writes trail reads)
            nc.vector.tensor_tensor(
                out=bnd[:, 0:ND], in0=bnd[:, 1:1 + ND], in1=bnd[:, 0:ND],
                op=mybir.AluOpType.subtract,
            )
            dt = bnd

            # ---------- phase A: fold groups (PE) -> axTaug rows 0..5 ----------
            # axT = sel^T @ dt + selfsel^T @ table   (self-loop term dis*x)
            CH = 512
            nchunks = (ND + CH - 1) // CH
            for c in range(nchunks):
                c0 = c * CH
                csz = min(CH, ND - c0)
                psA = psAp.tile([6, CH], f32, tag="psA")
                nc.tensor.matmul(
                    out=psA[:, :csz],
                    lhsT=sel_t[:],
                    rhs=dt[:, c0:c0 + csz],
                    start=True, stop=False,
                )
                nc.tensor.matmul(
                    out=psA[:, :csz],
                    lhsT=selfsel_t[:],
                    rhs=table[:, c0:c0 + csz],
                    start=False, stop=True,
                )
                nc.scalar.activation(
                    out=axTaug[0:6, c0:c0 + csz],
                    in_=psA[:, :csz],
                    func=mybir.ActivationFunctionType.Copy,
                )

            # ---------- phase B ----------
            # papt streamed as slabs of 12 node-tiles (1536 rows) per DMA.
            # dis[dst] is host-folded into papt columns, so relu needs no
            # per-partition scale and batches 4 node tiles wide.
            SLAB = 12
            QB = 4
            G_ps = psGp.tile([P128, P128], f32, tag="G")
            for s0 in range(0, NT, SLAB):
                ntiles = min(SLAB, NT - s0)
                r0 = s0 * 128
                nrows = ntiles * 128
                slab = pstream.tile([P128, SLAB * P128], f32, tag="papt")
                nc.sync.dma_start(
                    out=slab[:, 0:ntiles * P128].rearrange(
                        "p (u j) -> p u j", j=P128
                    ),
                    in_=papt[r0:r0 + nrows, :].rearrange(
                        "(u p) j -> p u j", p=128
                    ),
                )
                for q in range(0, ntiles, QB):
                    m = min(QB, ntiles - q)
                    ps1 = ps1p.tile([P128, QB * H], f32, tag="ps1")
                    for u in range(m):
                        t0 = (s0 + q + u) * 128
                        nc.tensor.matmul(
                            out=ps1[:, u * H:(u + 1) * H],
                            lhsT=axTaug[0:7, t0:t0 + 128],
                            rhs=w1_t[:],
                            start=True, stop=True,
                        )
                    h1 = hbuf.tile([P128, QB * H], f32, tag="h1")
                    nc.scalar.activation(
                        out=h1[:, :m * H], in_=ps1[:, :m * H],
                        func=mybir.ActivationFunctionType.Relu,
                    )
                    for u in range(m):
                        t = s0 + q + u
                        nc.tensor.matmul(
                            out=G_ps[:],
                            lhsT=slab[:, (q + u) * P128:(q + u + 1) * P128],
                            rhs=h1[:, u * H:(u + 1) * H],
                            start=(t == 0), stop=(t == NT - 1),
                        )

            G_sb = small.tile([P128, P128], f32)
            nc.scalar.activation(
                out=G_sb[:], in_=G_ps[:],
                func=mybir.ActivationFunctionType.Copy,
            )
            nc.sync.dma_start(out=gout[:], in_=G_sb[:])

    nc.compile()
    return nc


def _preprocess(x, edge_index, batch_idx):
    """Host-side integer/structure preprocessing. Returns per-core input maps
    (minus the device-computed parts) and head constants."""
    src = np.asarray(edge_index[0], dtype=np.int64)
    dst = np.asarray(edge_index[1], dtype=np.int64)
    loop = np.arange(N, dtype=np.int64)
    src2 = np.concatenate([src, loop])
    dst2 = np.concatenate([dst, loop])

    deg = np.bincount(dst2, minlength=N).astype(np.float32)  # >= 1
    dis = (1.0 / np.sqrt(deg)).astype(np.float32)
    sqdeg = np.sqrt(deg).astype(np.float32)

    bi = np.asarray(batch_idx, dtype=np.int64)
    cnt = np.bincount(bi, minlength=B).astype(np.float32)

    dis_pad = np.zeros(NPAD, np.float32)
    dis_pad[:N] = dis

    # dense PA = P @ A  [B, NPAD]
    w = (dis[src2] * dis[dst2]).astype(np.float64)
    flat = bi[dst2] * NPAD + src2
    PA = np.bincount(flat, weights=w, minlength=B * NPAD)
    PA = PA.reshape(B, NPAD).astype(np.float32)
    # pooling matrix P [B, NPAD]
    Pm = np.zeros((B, NPAD), np.float32)
    Pm[bi, np.arange(N)] = 1.0
    # fold dis[dst] into the pooled matrix columns: G uses relu(z) with
    # h1 = dis*relu(z), so papt rows get scaled by dis (exact: dis > 0).
    papt_full = (np.concatenate([PA, Pm], axis=0)
                 * dis_pad[None, :]).T.copy()  # [NPAD, 128]

    # per-(core, group) dst-sorted streams — REAL edges only; the appended
    # self-loops are handled analytically on device (dis^2 * x term).
    core = dst // NSHARD
    grp = src // NSHARD
    src_local = (src - grp * NSHARD).astype(np.int64)
    dst_local = (dst - core * NSHARD).astype(np.int64)
    cell = core * NG + grp
    key = cell * NSHARD + dst_local
    order = np.argsort(key, kind="stable")
    cell_s = cell[order]
    srcl_s = src_local[order]
    dstl_s = dst_local[order]
    cellcnt = np.bincount(cell_s, minlength=NG * NG)
    Jmax = int(cellcnt.max())
    JW = ((Jmax + 1 + 15) // 16) * 16
    assert JW <= 32768, JW

    cell_starts = np.zeros(NG * NG + 1, np.int64)
    np.cumsum(cellcnt, out=cell_starts[1:])

    gidx_all = np.full((NG, P128, JW // 16), NSHARD, np.int16)
    bidx_all = np.zeros((NG, P128, NB // 16), np.int16)
    for k in range(NG):
        for g in range(NG):
            ci = k * NG + g
            s0, s1 = cell_starts[ci], cell_starts[ci + 1]
            stream = np.full(JW, NSHARD, np.int64)
            stream[1:1 + (s1 - s0)] = srcl_s[s0:s1]
            gidx_all[k, 16 * g:16 * (g + 1)] = (
                stream.reshape(JW // 16, 16).T.astype(np.int16)
            )
            cnts = np.bincount(dstl_s[s0:s1], minlength=ND)
            bnd = np.cumsum(cnts)
            blist = np.zeros(NB, np.int64)
            blist[1:1 + ND] = bnd
            bidx_all[k, 16 * g:16 * (g + 1)] = (
                blist.reshape(NB // 16, 16).T.astype(np.int16)
            )

    # table-side constants
    x_np = np.asarray(x, dtype=np.float32)
    xt48 = np.zeros((48, NSHARD), np.float32)
    for g in range(NG):
        n0 = g * NSHARD
        n1 = min(n0 + NSHARD, N)
        if n1 > n0:
            xt48[6 * g:6 * g + 6, 0:n1 - n0] = x_np[n0:n1].T
    dis_tab = np.zeros((NG, NE), np.float32)
    dis_tab[:, :NSHARD] = dis_pad.reshape(NG, NSHARD)

    sq_pad = np.zeros(NPAD, np.float32)
    sq_pad[:N] = sqdeg

    sel = np.zeros((P128, 6), np.float32)
    for g in range(NG):
        for r in range(6):
            sel[16 * g + r, r] = 1.0
    # per-core self-loop selection: core k picks rows 16k+r of full=dis^2*x
    selfsel = np.zeros((NG, P128, 6), np.float32)
    for k in range(NG):
        for r in range(6):
            selfsel[k, 16 * k + r, r] = 1.0

    return {
        "JW": JW,
        "xt48": xt48,
        "dis_tab": dis_tab,
        "gidx_all": gidx_all,
        "bidx_all": bidx_all,
        "sq_pad": sq_pad,
        "papt_full": papt_full,
        "sel": sel,
        "selfsel": selfsel,
        "cnt": cnt,
    }


def _head(G, cnt, inputs):
    f = np.float32
    W2 = np.asarray(inputs["W2"], f)
    b2 = np.asarray(inputs["b2"], f)
    Wg = np.asarray(inputs["Wg"], f)
    bg = np.asarray(inputs["bg"], f)
    Et = np.asarray(inputs["Et"], f)
    Ek = np.asarray(inputs["Ek"], f)
    Ev = np.asarray(inputs["Ev"], f)
    Wp = np.asarray(inputs["Wp"], f)
    bp = np.asarray(inputs["bp"], f)
    Ekid = np.asarray(inputs["Ekid"], f)
    Wc = np.asarray(inputs["Wc"], f)
    bc = np.asarray(inputs["bc"], f)
    Wl = np.asarray(inputs["Wl"], f)
    bl = np.asarray(inputs["bl"], f)
    Wm1 = np.asarray(inputs["Wm1"], f)
    bm1 = np.asarray(inputs["bm1"], f)
    Wm2 = np.asarray(inputs["Wm2"], f)
    bm2 = np.asarray(inputs["bm2"], f)
    st = np.asarray(inputs["sol_type_idx"], np.int64)
    sk = np.asarray(inputs["sol_key_idx"], np.int64)
    sv = np.asarray(inputs["sol_val_idx"], np.int64)
    kid = np.asarray(inputs["kernel_id"], np.int64)
    cond = np.asarray(inputs["cond_vec"], f)
    loc = np.asarray(inputs["local_feats"], f)

    relu = lambda a: np.maximum(a, 0.0).astype(f)

    Ph2 = G[:B] @ W2 + cnt[:, None] * b2[None, :] + G[B:]
    g = (Ph2 / np.maximum(cnt, 1.0)[:, None]) @ Wg + bg

    seq_mean = np.concatenate(
        [Et[st].mean(axis=1), Ek[sk].mean(axis=1), Ev[sv].mean(axis=1)], axis=-1
    ).astype(f)
    p = relu(seq_mean @ Wp + bp)
    kvec = Ekid[kid]
    c = relu(cond @ Wc + bc)
    l = relu(loc @ Wl + bl)
    xf = np.concatenate([g, p, kvec, c, l], axis=1).astype(f)
    return (relu(xf @ Wm1 + bm1) @ Wm2 + bm2).astype(f)


def kernel(**inputs) -> np.ndarray:
    from concourse.bass_utils import run_bass_kernel_spmd

    pre = _preprocess(inputs["x"], inputs["edge_index"], inputs["batch_idx"])
    JW = pre["JW"]

    if JW not in _compiled:
        W1 = np.asarray(inputs["W1"], np.float32)
        b1 = np.asarray(inputs["b1"], np.float32)
        _compiled[JW] = _build_nc(JW)
    nc = _compiled[JW]

    W1 = np.asarray(inputs["W1"], np.float32)
    b1 = np.asarray(inputs["b1"], np.float32)
    w1aug = np.concatenate([W1, b1[None, :]], axis=0).astype(np.float32)  # [7,H]

    in_maps = []
    for k in range(NG):
        n0 = k * NSHARD
        in_maps.append({
            "xt48": pre["xt48"],
            "dis_tab": pre["dis_tab"],
            "gidx": pre["gidx_all"][k],
            "bidx": pre["bidx_all"][k],
            "sq": pre["sq_pad"][None, n0:n0 + NSHARD],
            "papt": np.ascontiguousarray(pre["papt_full"][n0:n0 + NSHARD]),
            "w1aug": w1aug,
            "sel": pre["sel"],
            "selfsel": pre["selfsel"][k],
            "zrow": np.zeros((1, NE), np.float32),
        })

    res = run_bass_kernel_spmd(nc, in_maps, core_ids=list(range(NG)))
    G = np.zeros((P128, P128), np.float64)
    for r in res.results:
        G += r["gout"].astype(np.float64)
    G = G.astype(np.float32)

    return _head(G, pre["cnt"], inputs)
